# revision 10
# baseline (speedup 1.0000x reference)
"""Distributed GCN (2-layer + readout) on 8 Trainium2 NeuronCores.

Src-sharded gather + one-hot-matmul aggregation + bf16 ReduceScatter:

Nodes are sharded 8-way by SRC owner (contiguous blocks of NSH=12544,
tile-aligned). Each GCN layer's linear W is folded to AFTER the
collective (A @ (hW) == (A @ h) @ W), so the gather table is just
u = dinv * h — elementwise, no matmul on the critical path before
gathering. Tables are stored as 256B rows (64 bf16 payload + 64 bf16
junk) to satisfy dma_gather's 256B-multiple elem constraint; gathers
are purely core-local (no AllGather).

Messages (edges) are sorted by dst and packed into 128-message columns
within tile-PAIRS (2x128 dst nodes); per pair the column count is the
max over cores so the instruction schedule is core-uniform. Each column
is aggregated with 1-2 TensorE matmuls: lhsT = a one-hot selection
matrix S (built on DVE via is_equal against an iota row; bf16) and
rhs = the gathered messages; PSUM accumulates per dst tile across a
bank of 8 tiles, then is copied (cast bf16) and spilled to a DRAM
accumulator. One bf16 ReduceScatter per layer replaces the baseline's
two f32 AllGathers; the received shard gets self-loop + @W + epilogue
via PE transposes (engines otherwise idle). This removes the combine
gather stage (~100k descriptors/layer) and the AllGather-before-gather
barrier that dominated the scatter/gather baseline.
"""
import numpy as np
import ml_dtypes

from concourse import bass, bacc, tile, mybir, bass_utils

F32 = mybir.dt.float32
BF16 = mybir.dt.bfloat16
I16 = mybir.dt.int16
I32 = mybir.dt.int32
NPBF16 = ml_dtypes.bfloat16

NCORES = 8
D = 64
CB = 32          # columns per gather block


def _roundup(x, m):
    return (x + m - 1) // m * m


def preprocess(edge_index, n_nodes):
    src = np.asarray(edge_index[0], dtype=np.int64)
    dst = np.asarray(edge_index[1], dtype=np.int64)
    E = len(src)

    NSH = _roundup((n_nodes + NCORES - 1) // NCORES, 128)  # shard size
    assert NSH * NCORES >= n_nodes and NSH < 32768
    NTOWN = NSH // 128            # own tiles per core
    NT = NCORES * NTOWN           # global dst tiles
    assert NT % 2 == 0
    NPAIR = NT // 2

    owner = src // NSH
    srow = src % NSH
    pair = dst // 256
    rel = dst - pair * 256        # [0, 256)

    deg = np.bincount(dst, minlength=NSH * NCORES).astype(np.float32) + 1.0
    deg_tiles = []
    for c in range(NCORES):
        d = np.ones(NSH, np.float32)
        lo = c * NSH
        d[:] = deg[lo:lo + NSH]
        d[max(0, n_nodes - lo):] = 1.0  # pad nodes
        deg_tiles.append(np.ascontiguousarray(d.reshape(NTOWN, 128).T))

    # per (core, pair) message lists, sorted by dst
    order = np.lexsort((dst, pair, owner))
    so, sp, sr, sl = owner[order], pair[order], srow[order], rel[order]
    # counts per core,pair
    cnt = np.zeros((NCORES, NPAIR), np.int64)
    np.add.at(cnt, (so, sp), 1)
    ncols = np.maximum(1, (cnt.max(axis=0) + 127) // 128)  # per pair
    col_base = np.concatenate([[0], np.cumsum(ncols)])
    NCOL = int(col_base[-1])
    NPOS = NCOL * 128

    # fill per-core vals (gather idx) and rel arrays
    vals = np.zeros((NCORES, NPOS), np.int16)
    rlo = np.full((NCORES, NPOS), -1.0, np.float32)
    # start offsets of each (core,pair) run inside the sorted arrays
    run_start = np.zeros((NCORES, NPAIR + 1), np.int64)
    csum = np.cumsum(cnt, axis=1)
    run_start[:, 1:] = csum
    core_off = np.concatenate([[0], np.cumsum(csum[:, -1])])[:-1]
    for c in range(NCORES):
        for k in range(NPAIR):
            a = core_off[c] + run_start[c, k]
            b = core_off[c] + run_start[c, k + 1]
            m = b - a
            if m == 0:
                continue
            q = np.arange(m) + col_base[k] * 128
            vals[c, q] = sr[a:b]
            rlo[c, q] = sl[a:b]

    # crossing flags per column (shared across cores)
    rel_by_col = rlo.reshape(NCORES, NCOL, 128)
    crossing = (rel_by_col >= 128.0).any(axis=(0, 2))
    # force a crossing col on pairs whose odd tile would otherwise never
    # be written (pair empty or no rel>=128 anywhere)
    for k in range(NPAIR):
        c0, c1 = col_base[k], col_base[k + 1]
        if not crossing[c0:c1].any():
            crossing[c1 - 1] = True
    xid_of = np.full(NCOL, -1, np.int64)
    xid_of[crossing] = np.arange(int(crossing.sum()))
    NX = int(crossing.sum())

    # compact hi-rel arrays
    rhi = np.full((NCORES, NX, 128), -1.0, np.float32)
    for c in range(NCORES):
        rb = rel_by_col[c][crossing]  # [NX, 128]
        hi = rb >= 128.0
        rhi[c][hi] = rb[hi] - 128.0

    # pack gather idx: [128, NPOS//16] int16, 16-partition wrap replicated
    gidx_all, rlo_all, rhi_all = [], [], []
    for c in range(NCORES):
        gidx_all.append(np.ascontiguousarray(
            np.tile(vals[c].reshape(-1, 16).T, (8, 1))))
        rlo_all.append(np.ascontiguousarray(
            rel_by_col[c].T).astype(NPBF16))     # [128, NCOL]
        if NX:
            rhi_all.append(np.ascontiguousarray(
                rhi[c].T).astype(NPBF16))        # [128, NX]
        else:
            rhi_all.append(np.zeros((128, 1), NPBF16))

    # column schedule (shared): for each column: pair k, first/last col
    # flags, xid (or -1). The pair's matmuls form ONE psum accumulation
    # group (2KB zero region = bank): start on first col's even matmul,
    # stop on the last emitted matmul of the pair.
    cols = []
    for k in range(NPAIR):
        c0, c1 = col_base[k], col_base[k + 1]
        for c in range(c0, c1):
            last = c == c1 - 1
            xid = int(xid_of[c]) if crossing[c] else -1
            cols.append(dict(
                k=k, first=(c == c0), last=last, xid=xid,
                stop_even=(last and xid < 0),
                stop_odd=(last and xid >= 0),
            ))

    meta = dict(NSH=NSH, NTOWN=NTOWN, NT=NT, NPAIR=NPAIR, NCOL=NCOL,
                NPOS=NPOS, NX=max(NX, 1), cols=cols, n_nodes=n_nodes)
    return meta, gidx_all, rlo_all, rhi_all, deg_tiles


def build(meta):
    NSH, NTOWN, NT = meta["NSH"], meta["NTOWN"], meta["NT"]
    NCOL, NPOS, NX = meta["NCOL"], meta["NPOS"], meta["NX"]
    cols = meta["cols"]

    nc = bacc.Bacc("TRN2", target_bir_lowering=False, debug=False,
                   num_devices=NCORES, num_swdge_queues=4)

    xse = nc.dram_tensor("xs", [128, NTOWN * D], F32, kind="ExternalInput")
    dege = nc.dram_tensor("deg", [128, NTOWN], F32, kind="ExternalInput")
    gidxe = nc.dram_tensor("gidx", [128, NPOS // 16], I16,
                           kind="ExternalInput")
    rloe = nc.dram_tensor("rlo", [128, NCOL], BF16, kind="ExternalInput")
    rhie = nc.dram_tensor("rhi", [128, NX], BF16, kind="ExternalInput")
    W1e = nc.dram_tensor("W1", [D, D], BF16, kind="ExternalInput")
    W2e = nc.dram_tensor("W2", [D, D], BF16, kind="ExternalInput")
    b1e = nc.dram_tensor("b1bc", [128, D], F32, kind="ExternalInput")
    b2e = nc.dram_tensor("b2bc", [128, D], F32, kind="ExternalInput")
    woute = nc.dram_tensor("woutbc", [128, D], F32, kind="ExternalInput")
    boute = nc.dram_tensor("boutbc", [128, 1], F32, kind="ExternalInput")
    idente = nc.dram_tensor("identbf", [128, 128], BF16,
                            kind="ExternalInput")
    oute = nc.dram_tensor("out", [128, NTOWN], F32, kind="ExternalOutput")

    T0 = nc.dram_tensor("T0", [NSH, 128], BF16)
    T1 = nc.dram_tensor("T1", [NSH, 128], BF16)
    acc = nc.dram_tensor("acc", [NT * 128, D], BF16)
    shard = [nc.dram_tensor(f"shard{L}", [NSH, D], BF16)
             for L in (0, 1)]

    def nodemaj(dram, g):
        # DRAM [(g p), d] viewed as [128, g, d]
        return dram.ap().rearrange("(g p) d -> p g d", p=128)

    with tile.TileContext(nc) as tc:
        with (
            tc.tile_pool(name="const", bufs=1) as pool,
            tc.tile_pool(name="msg", bufs=3) as msgpool,
            tc.tile_pool(name="slo", bufs=2) as slopool,
            tc.tile_pool(name="shi", bufs=2) as shipool,
            tc.tile_pool(name="stage", bufs=4) as stagepool,
            tc.tile_pool(name="aggT", bufs=2) as aggTpool,
            tc.tile_pool(name="hg", bufs=2) as hgpool,
            tc.tile_pool(name="aggps", bufs=4, space="PSUM") as aggps,
            tc.tile_pool(name="trps", bufs=2, space="PSUM") as trps,
            tc.tile_pool(name="mmps", bufs=2, space="PSUM") as mmps,
        ):
            # ---- constants ----
            gidx_t = pool.tile([128, NPOS // 16], I16, tag="gidx")
            rlo_t = pool.tile([128, NCOL], BF16, tag="rlo")
            rhi_t = pool.tile([128, NX], BF16, tag="rhi")
            nc.scalar.dma_start(out=gidx_t[:], in_=gidxe[:])
            nc.scalar.dma_start(out=rlo_t[:], in_=rloe[:])
            nc.scalar.dma_start(out=rhi_t[:], in_=rhie[:])
            W1_t = pool.tile([D, D], BF16, tag="w1")
            W2_t = pool.tile([D, D], BF16, tag="w2")
            b1_t = pool.tile([128, D], F32, tag="b1")
            b2_t = pool.tile([128, D], F32, tag="b2")
            wout_t = pool.tile([128, D], F32, tag="wout")
            bout_t = pool.tile([128, 1], F32, tag="bout")
            ident_t = pool.tile([128, 128], BF16, tag="ident")
            nc.scalar.dma_start(out=W1_t[:], in_=W1e[:])
            nc.scalar.dma_start(out=W2_t[:], in_=W2e[:])
            nc.scalar.dma_start(out=b1_t[:], in_=b1e[:])
            nc.scalar.dma_start(out=b2_t[:], in_=b2e[:])
            nc.scalar.dma_start(out=wout_t[:], in_=woute[:])
            nc.scalar.dma_start(out=bout_t[:], in_=boute[:])
            nc.scalar.dma_start(out=ident_t[:], in_=idente[:])
            deg_t = pool.tile([128, NTOWN], F32, tag="deg")
            nc.sync.dma_start(out=deg_t[:], in_=dege[:])
            dinv_t = pool.tile([128, NTOWN], F32, tag="dinv")
            nc.scalar.activation(dinv_t[:], deg_t[:],
                                 mybir.ActivationFunctionType.Sqrt)
            nc.vector.reciprocal(dinv_t[:], dinv_t[:])
            iota_i = pool.tile([128, 128], I32, tag="iotai")
            iota_t = pool.tile([128, 128], BF16, tag="iota")
            nc.gpsimd.iota(iota_i[:], pattern=[[1, 128]], base=0,
                           channel_multiplier=0)
            nc.vector.tensor_copy(iota_t[:], iota_i[:])

            xs_t = pool.tile([128, NTOWN, D], F32, tag="xs")
            nc.sync.dma_start(
                out=xs_t[:],
                in_=xse.ap().rearrange("p (g d) -> p g d", d=D))

            # ---- u0 = dinv * x into junk-row staging ----
            tjunk = pool.tile([128, NTOWN, 128], BF16, tag="tjunk")
            nc.vector.memset(tjunk[:], 0.0)
            dvb = dinv_t[:].unsqueeze(2).broadcast_to([128, NTOWN, D])
            nc.vector.tensor_tensor(tjunk[:, :, 0:D], xs_t[:], dvb,
                                    mybir.AluOpType.mult)
            nc.sync.dma_start(out=nodemaj(T0, NTOWN), in_=tjunk[:])

            NBLK = (NCOL + CB - 1) // CB

            def agg_layer(L):
                Tbl = T0 if L == 0 else T1
                psum_live = {}
                st_cur = [None]
                ci = 0
                for blk in range(NBLK):
                    c0 = blk * CB
                    bc = min(CB, NCOL - c0)
                    mt = msgpool.tile([128, CB, 128], BF16, tag="m")
                    nc.gpsimd.dma_gather(
                        mt[:, :bc, :], Tbl[:],
                        gidx_t[:, c0 * 8:(c0 + bc) * 8],
                        num_idxs=bc * 128, num_idxs_reg=bc * 128,
                        elem_size=128, single_packet=False,
                        queue_num=blk % 4)
                    sl = slopool.tile([128, CB, 128], BF16, tag="sl")
                    nc.vector.tensor_tensor(
                        sl[:, :bc, :],
                        rlo_t[:, c0:c0 + bc].unsqueeze(2).broadcast_to(
                            [128, bc, 128]),
                        iota_t[:].unsqueeze(1).broadcast_to([128, bc, 128]),
                        mybir.AluOpType.is_equal)
                    # crossing cols in this block -> compact hi one-hots
                    xids = [cols[c]["xid"] for c in range(c0, c0 + bc)
                            if cols[c]["xid"] >= 0]
                    if xids:
                        x0, nxb = xids[0], len(xids)
                        assert xids == list(range(x0, x0 + nxb))
                        sh = shipool.tile([128, CB, 128], BF16, tag="sh")
                        nc.vector.tensor_tensor(
                            sh[:, :nxb, :],
                            rhi_t[:, x0:x0 + nxb].unsqueeze(2).broadcast_to(
                                [128, nxb, 128]),
                            iota_t[:].unsqueeze(1).broadcast_to(
                                [128, nxb, 128]),
                            mybir.AluOpType.is_equal)
                    for c in range(c0, c0 + bc):
                        info = cols[c]
                        k = info["k"]
                        if k not in psum_live:
                            psum_live[k] = aggps.tile(
                                [128, 2, 64], F32, tag="agg", name="aggp")
                        pt = psum_live[k]
                        j = c - c0
                        nc.tensor.matmul(
                            pt[:, 0, :], sl[:, j, :], mt[:, j, 0:D],
                            start=info["first"], stop=info["stop_even"])
                        if info["xid"] >= 0:
                            xj = info["xid"] - x0
                            nc.tensor.matmul(
                                pt[:, 1, :], sh[:, xj, :], mt[:, j, 0:D],
                                start=False, stop=info["stop_odd"])
                        if info["last"]:
                            slot = k % 4
                            if slot == 0:
                                st_cur[0] = stagepool.tile(
                                    [128, 8, 64], BF16, tag="st",
                                    name="stg")
                            nc.vector.tensor_copy(
                                st_cur[0][:, 2 * slot:2 * slot + 2, :],
                                pt[:])
                            del psum_live[k]
                            if slot == 3 or k == meta["NPAIR"] - 1:
                                t0g = (k // 4) * 8
                                nc.sync.dma_start(
                                    out=acc[t0g * 128:(t0g + 8) * 128, :]
                                    .rearrange("(g p) d -> p g d", p=128),
                                    in_=st_cur[0][:])
                        ci += 1
                assert not psum_live, psum_live.keys()

            def post_layer(L):
                """RS result -> agg_u -> @W -> epilogue."""
                sh_t = pool.tile([128, NTOWN, D], BF16, tag="shards")
                nc.sync.dma_start(out=sh_t[:], in_=nodemaj(shard[L], NTOWN))
                # add self-loop term u_own (in tjunk payload half)
                nc.vector.tensor_tensor(sh_t[:], sh_t[:],
                                        tjunk[:, :, 0:D],
                                        mybir.AluOpType.add)
                W_t = W1_t if L == 0 else W2_t
                b_t = b1_t if L == 0 else b2_t
                if L == 1:
                    o_t = pool.tile([128, NTOWN], F32, tag="o")
                for tg in range(0, NTOWN, 8):
                    ng = min(8, NTOWN - tg)
                    tp = trps.tile([64, 8, 128], BF16, tag="tr")
                    for t in range(tg, tg + ng):
                        nc.tensor.transpose(tp[:, t - tg, :], sh_t[:, t, :],
                                            ident_t[:])
                    aT = aggTpool.tile([64, 8, 128], BF16, tag="aT")
                    nc.vector.tensor_copy(aT[:, :ng, :], tp[:, :ng, :])
                    mp = mmps.tile([128, 8, 64], F32, tag="mm")
                    for t in range(tg, tg + ng):
                        nc.tensor.matmul(mp[:, t - tg, :], aT[:, t - tg, :],
                                         W_t[:])
                    hg = hgpool.tile([128, 8, 64], F32, tag="hg")
                    dvg = dinv_t[:, tg:tg + ng].unsqueeze(2).broadcast_to(
                        [128, ng, 64])
                    bbg = b_t[:].unsqueeze(1).broadcast_to([128, ng, 64])
                    nc.vector.tensor_tensor(hg[:, :ng, :], mp[:, :ng, :],
                                            dvg, mybir.AluOpType.mult)
                    nc.vector.tensor_tensor(hg[:, :ng, :], hg[:, :ng, :],
                                            bbg, mybir.AluOpType.add)
                    nc.scalar.activation(hg[:, :ng, :], hg[:, :ng, :],
                                         mybir.ActivationFunctionType.Relu)
                    if L == 0:
                        # u1 = dinv * h into table staging payload
                        nc.vector.tensor_tensor(
                            tjunk[:, tg:tg + ng, 0:D], hg[:, :ng, :], dvg,
                            mybir.AluOpType.mult)
                    else:
                        wbg = wout_t[:].unsqueeze(1).broadcast_to(
                            [128, ng, 64])
                        nc.vector.tensor_tensor(hg[:, :ng, :], hg[:, :ng, :],
                                                wbg, mybir.AluOpType.mult)
                        nc.vector.tensor_reduce(
                            o_t[:, tg:tg + ng], hg[:, :ng, :],
                            axis=mybir.AxisListType.X,
                            op=mybir.AluOpType.add)
                if L == 0:
                    nc.sync.dma_start(out=nodemaj(T1, NTOWN), in_=tjunk[:])
                else:
                    nc.vector.tensor_scalar_add(o_t[:], o_t[:], bout_t[:])
                    nc.sync.dma_start(out=oute[:], in_=o_t[:])

            for L in (0, 1):
                agg_layer(L)
                nc.gpsimd.collective_compute(
                    "ReduceScatter", mybir.AluOpType.add,
                    replica_groups=[list(range(NCORES))],
                    ins=[acc.ap().opt()],
                    outs=[shard[L].ap().opt()])
                post_layer(L)

    nc.compile()
    return nc


_CACHE = {}


def kernel(x, edge_index, batch, W1, b1, W2, b2, Wout, bout, _trace=False):
    x = np.asarray(x, np.float32)
    edge_index = np.asarray(edge_index)
    W1 = np.asarray(W1, np.float32)
    W2 = np.asarray(W2, np.float32)
    b1 = np.asarray(b1, np.float32)
    b2 = np.asarray(b2, np.float32)
    Wout = np.asarray(Wout, np.float32)
    bout = np.asarray(bout, np.float32).reshape(-1)
    N = x.shape[0]

    key = (N, edge_index.shape[1])
    if key not in _CACHE:
        meta, gidx_all, rlo_all, rhi_all, deg_tiles = preprocess(
            edge_index, N)
        nc = build(meta)
        _CACHE[key] = (meta, gidx_all, rlo_all, rhi_all, deg_tiles, nc)
    meta, gidx_all, rlo_all, rhi_all, deg_tiles, nc = _CACHE[key]
    NSH, NTOWN = meta["NSH"], meta["NTOWN"]

    identbf = np.eye(128, dtype=np.float32).astype(NPBF16)
    b1bc = np.tile(b1[None, :], (128, 1)).astype(np.float32)
    b2bc = np.tile(b2[None, :], (128, 1)).astype(np.float32)
    woutbc = np.tile(Wout.reshape(1, -1), (128, 1)).astype(np.float32)
    boutbc = np.full((128, 1), float(bout[0]), np.float32)
    W1bf = W1.astype(NPBF16)
    W2bf = W2.astype(NPBF16)

    in_maps = []
    for c in range(NCORES):
        xsh = np.zeros((NSH, D), np.float32)
        lo, hi = c * NSH, min((c + 1) * NSH, N)
        if hi > lo:
            xsh[:hi - lo] = x[lo:hi]
        # node-major [128, NTOWN, D]: xs[p, g, :] = x[lo + g*128 + p]
        xs = np.ascontiguousarray(
            xsh.reshape(NTOWN, 128, D).transpose(1, 0, 2).reshape(
                128, NTOWN * D))
        in_maps.append({
            "xs": xs, "deg": deg_tiles[c],
            "gidx": gidx_all[c], "rlo": rlo_all[c], "rhi": rhi_all[c],
            "W1": W1bf, "W2": W2bf, "b1bc": b1bc, "b2bc": b2bc,
            "woutbc": woutbc, "boutbc": boutbc, "identbf": identbf,
        })

    res = bass_utils.run_bass_kernel_spmd(
        nc, in_maps, core_ids=list(range(NCORES)), trace=_trace)

    out = np.zeros(N, np.float32)
    for c in range(NCORES):
        o = res.results[c]["out"]  # [128, NTOWN]
        arr = o.T.ravel()          # node-major: g*128 + p
        lo, hi = c * NSH, min((c + 1) * NSH, N)
        if hi > lo:
            out[lo:hi] = arr[:hi - lo]
    if _trace:
        return out, res.exec_time_ns
    return out
